# revision 5
# baseline (speedup 1.0000x reference)
"""Trainium2 Bass kernel for a differentiable addressing head (NTM-style), v4.

v4 = v2 structure with a mixed-precision memory stream:
  - ~69% of memory (11 of 16 half-batch stripes) is sent as fp8 e3m4
    (4-bit mantissa, range +-15.5 — ideal for randn data), the rest as
    bf16.  DMA bytes drop 16.8 MB -> 11.0 MB per core.
  - Squares are no longer in-place: each stripe is squared into a
    rotating pool of bf16 tiles (dst-based), so squares start at DMA
    arrival and stripes stay pristine.
  - Engine split per stripe is chosen greedily: ACT is dtype-agnostic
    (~131 G elem/s), DVE runs 2x on bf16 (~229 G/s) but 1x on fp8
    (~118 G/s); bf16 stripes are placed late (batches 5-7) so DVE can
    drain them fast.
  - dot MMs stream fp8/bf16 rhs against bf16 key strips (mixed-dtype
    matmul is allowed for non-fp32); norm MMs stream the bf16 squares.

Sharding: data-parallel over batch across 8 cores (8 batches/core).
Self-contained: hardcodes shapes B=64, N=8192, D=128, C=256.
"""

import os
import sys

import numpy as np

for _p in ("/opt/trn_rl_repo", "/opt/pypackages"):
    if _p not in sys.path and os.path.isdir(_p):
        sys.path.insert(0, _p)

import concourse.bacc as bacc
import concourse.bass as bass
import concourse.tile as tile
from concourse import mybir
from concourse.bass_utils import run_bass_kernel_spmd

F32 = mybir.dt.float32
BF16 = mybir.dt.bfloat16
F16 = mybir.dt.float16
F8E3 = mybir.dt.float8e3
AF = mybir.ActivationFunctionType
OP = mybir.AluOpType

B, N, D, C = 64, 8192, 128, 256
NCORES = 8
BL = B // NCORES          # batches per core = 8
NW = 16                   # 512-wide windows per batch row
W = N // NW               # 512, window width (= light-tile free dim)
H = 4096                  # half-batch stripe width
EPS = 1e-8

# Stripe program order: UNIFORM half-batch [128, 4096] stripes (fp8 4KB
# rows, bf16 8KB rows) in batch-pair round-robin so the 4 PSUM col-groups
# stream concurrently.  Uniform row sizes matter: transfers with 16KB or
# 2KB rows deal descriptors unevenly across the 16 SDMA engines, and the
# straggler engine delays every stripe-completion semaphore behind it.
# fp8e3 (~69%): batches 0-4 + b7 second half (small fp8 drain stripe
# last); bf16: b5, b6, b7 first half (DVE squares bf16 at 2x).
# Entry: (batch, col0, col1, kind, src_idx);  kind: 0 = fp8e3 (m8
# [11,D,H]), 1 = bf16 (mb [5,D,H])
STRIPES = [
    (0, 0, 4096, 0, 0),
    (2, 0, 4096, 0, 1),
    (4, 0, 4096, 0, 2),
    (6, 0, 4096, 1, 0),
    (1, 0, 4096, 0, 3),
    (3, 0, 4096, 0, 4),
    (5, 0, 4096, 1, 1),
    (7, 0, 4096, 1, 2),
    (0, 4096, 8192, 0, 5),
    (2, 4096, 8192, 0, 6),
    (4, 4096, 8192, 0, 7),
    (6, 4096, 8192, 1, 3),
    (1, 4096, 8192, 0, 8),
    (3, 4096, 8192, 0, 9),
    (5, 4096, 8192, 1, 4),
    (7, 4096, 8192, 0, 10),
]

# partition shift masks (within each 16-partition batch group)
MASK_L = [(i % 16 + 1) % 16 + 16 * (i // 16) for i in range(32)]   # out[p]=in[p+1]
MASK_R = [(i % 16 - 1) % 16 + 16 * (i // 16) for i in range(32)]   # out[p]=in[p-1]

_NC = None
PROFILE = False
LAST_RESULTS = None


def _pack_consts():
    cols = {}
    off = 0

    def reserve(name, width):
        nonlocal off
        cols[name] = (off, off + width)
        off += width

    reserve("csT0", BL)
    reserve("csT1", BL)
    reserve("Wk0", D)
    reserve("Wk1", D)
    global PACKA_W
    PACKA_W = off
    reserve("Wc0", 6)
    reserve("Wc1", 6)
    reserve("indT", 128)      # rows 0..7 hold indT (BL x 128)
    reserve("ones_col", 1)
    reserve("ones_strip", 63)
    reserve("P", 128)         # same-batch indicator
    reserve("bias6", 6)       # rows 0..7
    reserve("eps_col", 1)
    return cols, off


PACKA_W = 0
PACK_COLS, PACK_W = _pack_consts()


def _patch_act_tables():
    if getattr(bacc, "_act_tables_patched", False):
        return
    orig = bacc.get_activation_tables

    def filtered(module_arch):
        t = orig(module_arch)
        pref = "natural_log_exp_and_others"
        if pref in t:
            mine = {
                AF.Exp, AF.Ln, AF.Square, AF.Copy, AF.Identity, AF.MemsetZero
            } & t[pref]
            for k in t:
                if k != pref:
                    t[k] = t[k] - mine
        return t

    bacc.get_activation_tables = filtered
    bacc._act_tables_patched = True


def build_nc():
    _patch_act_tables()
    nc = bacc.Bacc()

    m8_d = nc.dram_tensor("m8", [11, D, H], F8E3, kind="ExternalInput")
    mb_d = nc.dram_tensor("mb", [5, D, H], BF16, kind="ExternalInput")
    packA_d = nc.dram_tensor("packA", [128, PACKA_W], F32, kind="ExternalInput")
    pack_d = nc.dram_tensor("packB", [128, PACK_W - PACKA_W], F32, kind="ExternalInput")
    # pw/out pre-reshaped to [128, W] on host: a [BL, N] layout would DMA
    # through few fat descriptors that serialize on one SDMA engine.
    pw_d = nc.dram_tensor("pw", [128, W], F16, kind="ExternalInput")
    out_d = nc.dram_tensor("out", [128, W], F32, kind="ExternalOutput")

    with tile.TileContext(nc) as tc:
        with (
            tc.tile_pool(name="const", bufs=1) as cp,
            tc.tile_pool(name="mem", bufs=1) as memp,
            tc.tile_pool(name="sq", bufs=1) as sqp,
            tc.tile_pool(name="light", bufs=1) as lp,
            tc.tile_pool(name="psmm", bufs=1, space="PSUM") as psA,
            tc.tile_pool(name="pstiny", bufs=2, space="PSUM") as psB,
        ):
            # ---- all DMA issues up-front (sync engine, no waits) ----
            stripes = []

            def stripe_dma(i):
                b, c0, c1, kind, sidx = STRIPES[i]
                cols = c1 - c0
                dt_ = F8E3 if kind == 0 else BF16
                st = memp.tile([128, cols], dt_, name=f"mst_{i}")
                src = (m8_d if kind == 0 else mb_d)[:]
                nc.sync.dma_start(st[:], src[sidx])
                stripes.append(st)

            packA_raw = cp.tile([128, PACKA_W], F32, name="packA_raw")
            nc.sync.dma_start(packA_raw[:], packA_d[:])  # key path first
            stripe_dma(0)
            pack_raw = cp.tile([128, PACK_W - PACKA_W], F32, name="pack_raw")
            nc.sync.dma_start(pack_raw[:], pack_d[:])
            pw_sb = cp.tile([128, W], F16, name="pw_sb")
            nc.sync.dma_start(pw_sb[:], pw_d[:])
            for i in range(1, len(STRIPES)):
                stripe_dma(i)

            # ---- gpsimd staging of constants (single producer for PE) ----
            def gslice(name, rows=128, dtype=F32):
                a, bb_ = PACK_COLS[name]
                g = cp.tile([rows, bb_ - a], dtype, name=f"g_{name}")
                if bb_ <= PACKA_W:
                    nc.gpsimd.tensor_copy(g[:], packA_raw[0:rows, a:bb_])
                else:
                    nc.gpsimd.tensor_copy(
                        g[:], pack_raw[0:rows, a - PACKA_W : bb_ - PACKA_W])
                return g

            # key path staged via ONE fast DVE copy (gpsimd is ~24 G elem/s;
            # DVE does this 272-col block in ~250 ns) so the key matmuls
            # and strips are ready ~2 us earlier
            g_key = cp.tile([128, PACKA_W], F32, name="g_key")
            nc.vector.tensor_copy(g_key[:], packA_raw[:])
            csT0 = g_key[:, PACK_COLS["csT0"][0]:PACK_COLS["csT0"][1]]
            csT1 = g_key[:, PACK_COLS["csT1"][0]:PACK_COLS["csT1"][1]]
            Wk0 = g_key[:, PACK_COLS["Wk0"][0]:PACK_COLS["Wk0"][1]]
            Wk1 = g_key[:, PACK_COLS["Wk1"][0]:PACK_COLS["Wk1"][1]]
            # key projection ASAP (strips gate the first dot matmuls)
            key_ps = psB.tile([128, BL], F32, tag="tiny")
            nc.tensor.matmul(key_ps[:], lhsT=Wk0[:], rhs=csT0[:], start=True, stop=False)
            nc.tensor.matmul(key_ps[:], lhsT=Wk1[:], rhs=csT1[:], start=False, stop=True)
            keyT = cp.tile([128, BL], F32, name="keyT")
            nc.vector.tensor_copy(keyT[:], key_ps[:])
            # zero-padded key strips: strips[:, b, 31] = keyT[:, b]
            strips = cp.tile([128, BL, 63], BF16, name="strips")
            nc.vector.memset(strips[:], 0.0)
            for b in range(BL):
                nc.vector.tensor_copy(strips[:, b, 31:32], keyT[:, b : b + 1])

            eps_col = gslice("eps_col")
            g_wc = cp.tile([128, 12], F32, name="g_wc")
            nc.gpsimd.tensor_copy(g_wc[:], pack_raw[:, 0:12])
            Wc0 = g_wc[:, 0:6]
            Wc1 = g_wc[:, 6:12]
            indT = gslice("indT", rows=BL)
            ones_col = gslice("ones_col")
            ones_strip = gslice("ones_strip", dtype=BF16)
            bias6 = gslice("bias6", rows=BL)
            P_sb = gslice("P")  # staged LAST: absorbing its tick covers all

            # absorber: advance PE's observed DVE tick (strips) before the
            # heavy matmuls
            absorb = psB.tile([128, 2], F32, tag="absorb", bufs=1, name="absorb")
            nc.tensor.matmul(absorb[0:8, 0:1], lhsT=strips[:, :, 31],
                             rhs=strips[:, 0, 31:32], start=True, stop=True,
                             skip_group_check=True)

            FB = lp.tile([128, 7], F32, name="FB")
            F_beta = FB[:, 0:1]
            F_gate = FB[:, 1:2]
            F_s0 = FB[:, 2:3]
            F_s1 = FB[:, 3:4]
            F_s2 = FB[:, 4:5]
            F_gamma = FB[:, 5:6]
            F_kn2 = lp.tile([128, 1], F32, name="F_kn2")
            PWc = lp.tile([128, W], F16, name="PWc")
            F_gs0 = lp.tile([128, 1], F32, name="F_gs0")
            F_gs1 = lp.tile([128, 1], F32, name="F_gs1")
            F_gs2 = lp.tile([128, 1], F32, name="F_gs2")

            def emit_proj_block():
                proj_ps = psB.tile([BL, 6], F32, tag="tiny")
                nc.tensor.matmul(proj_ps[:], lhsT=csT0[:], rhs=Wc0[:], start=True, stop=False)
                nc.tensor.matmul(proj_ps[:], lhsT=csT1[:], rhs=Wc1[:], start=False, stop=True)
                proj = lp.tile([BL, 6], F32, name="proj")
                nc.vector.tensor_add(proj[:], proj_ps[:], bias6[:])
                kq = lp.tile([128, BL], F32, name="kq")
                nc.scalar.activation(kq[:], keyT[:], AF.Square)
                kn2_ps = psB.tile([BL, 1], F32, tag="tiny")
                nc.tensor.matmul(kn2_ps[:], lhsT=kq[:], rhs=ones_col[:], start=True, stop=True)
                kn2 = lp.tile([BL, 1], F32, name="kn2")
                nc.vector.tensor_copy(kn2[:], kn2_ps[:])
                kn2F_ps = psB.tile([128, 1], F32, tag="tiny")
                nc.tensor.matmul(kn2F_ps[:], lhsT=indT[:], rhs=kn2[:], start=True, stop=True)
                nc.vector.tensor_copy(F_kn2[:], kn2F_ps[:])
                scal = lp.tile([BL, 7], F32, name="scal")
                # softplus(x) = ln(1 + exp(x)); beta = softplus + 1
                eb = lp.tile([BL, 1], F32, name="eb")
                nc.scalar.activation(eb[:], proj[:, 0:1], AF.Exp)
                sp_b = lp.tile([BL, 1], F32, name="sp_b")
                nc.scalar.activation(sp_b[:], eb[:], AF.Ln, bias=1.0)
                nc.vector.tensor_scalar_add(scal[:, 0:1], sp_b[:], 1.0)
                # gate = sigmoid(x)
                eg = lp.tile([BL, 1], F32, name="eg")
                nc.scalar.activation(eg[:], proj[:, 1:2], AF.Exp, scale=-1.0)
                dg = lp.tile([BL, 1], F32, name="dg")
                nc.vector.tensor_scalar_add(dg[:], eg[:], 1.0)
                nc.vector.reciprocal(scal[:, 1:2], dg[:])
                # shift softmax numerator straight into scal; 1/sum goes
                # to col 6 and is folded into the coefficient muls below
                nc.scalar.activation(scal[:, 2:5], proj[:, 2:5], AF.Exp)
                ssum = lp.tile([BL, 1], F32, name="ssum")
                nc.vector.reduce_sum(ssum[:], scal[:, 2:5], axis=mybir.AxisListType.X)
                nc.vector.reciprocal(scal[:, 6:7], ssum[:])
                # gamma = softplus(z) + 1
                egm = lp.tile([BL, 1], F32, name="egm")
                nc.scalar.activation(egm[:], proj[:, 5:6], AF.Exp)
                sp_g = lp.tile([BL, 1], F32, name="sp_g")
                nc.scalar.activation(sp_g[:], egm[:], AF.Ln, bias=1.0)
                nc.vector.tensor_scalar_add(scal[:, 5:6], sp_g[:], 1.0)
                FB_ps = psB.tile([128, 7], F32, tag="tiny")
                nc.tensor.matmul(FB_ps[:], lhsT=indT[:], rhs=scal[:], start=True, stop=True)
                nc.vector.tensor_copy(FB[:], FB_ps[:])
                F_g1 = lp.tile([128, 1], F32, name="F_g1")
                nc.vector.tensor_scalar(F_g1[:], F_gate, -1.0, 1.0, op0=OP.mult, op1=OP.add)
                F_rs = FB[:, 6:7]
                F_g1r = lp.tile([128, 1], F32, name="F_g1r")
                nc.vector.tensor_mul(F_g1r[:], F_g1[:], F_rs)
                F_gr = lp.tile([128, 1], F32, name="F_gr")
                nc.vector.tensor_mul(F_gr[:], F_gate, F_rs)
                F_s0g = lp.tile([128, 1], F32, name="F_s0g")
                nc.vector.tensor_mul(F_s0g[:], F_s0, F_g1r[:])
                F_s1g = lp.tile([128, 1], F32, name="F_s1g")
                nc.vector.tensor_mul(F_s1g[:], F_s1, F_g1r[:])
                F_s2g = lp.tile([128, 1], F32, name="F_s2g")
                nc.vector.tensor_mul(F_s2g[:], F_s2, F_g1r[:])
                # gate-scaled shift coefficients for conv(E)
                nc.vector.tensor_mul(F_gs0[:], F_s0, F_gr[:])
                nc.vector.tensor_mul(F_gs1[:], F_s1, F_gr[:])
                nc.vector.tensor_mul(F_gs2[:], F_s2, F_gr[:])
                nc.vector.tensor_scalar_mul(PWc[:], pw_sb[:], F_s1g[:])
                nc.vector.scalar_tensor_tensor(
                    PWc[:, 0 : W - 1], pw_sb[:, 1:W], F_s0g[:], PWc[:, 0 : W - 1],
                    op0=OP.mult, op1=OP.add)
                nc.vector.scalar_tensor_tensor(
                    PWc[:, 1:W], pw_sb[:, 0 : W - 1], F_s2g[:], PWc[:, 1:W],
                    op0=OP.mult, op1=OP.add)
                pwL = lp.tile([128, 1], F16, name="pwL")
                nc.vector.stream_shuffle(pwL[:], pw_sb[:, 0:1], MASK_L)
                pwR = lp.tile([128, 1], F16, name="pwR")
                nc.vector.stream_shuffle(pwR[:], pw_sb[:, W - 1 : W], MASK_R)
                nc.vector.scalar_tensor_tensor(
                    PWc[:, W - 1 : W], pwL[:], F_s0g[:], PWc[:, W - 1 : W],
                    op0=OP.mult, op1=OP.add)
                nc.vector.scalar_tensor_tensor(
                    PWc[:, 0:1], pwR[:], F_s2g[:], PWc[:, 0:1],
                    op0=OP.mult, op1=OP.add)

            # ---- heavy phase ----
            dotPs = psA.tile([128, W], F32, tag="dotP", name="dotP")
            nrmPs = psA.tile([128, W], F32, tag="nrmP", name="nrmP")

            # square engine balancing on SIMULATED CLOCKS (HW-calibrated).
            # ACT is dtype-agnostic (w+352)/1.2; DVE is 2x on bf16 (all-2B
            # operands) but 1x on fp8 input: (w_eff+65)/0.96.  Each
            # engine's clock advances from max(clock, stripe arrival), so
            # the split accounts for both load AND when data lands.
            act_clk = [11500.0]   # table load ends ~8.5us + first waits
            dve_clk = [12000.0]   # strips memset/copies
            arr = []              # stripe arrival estimates (ns)
            _t = 9300.0
            for _b, _c0, _c1, _k, _s in STRIPES:
                _t += (_c1 - _c0) * (2 if _k else 1) * 128 / 353.0
                arr.append(_t)

            def a_cost(w):
                return 0.0 if w == 0 else (w + 352) / 1.27 + 30

            def d_cost(w, kind):
                if w == 0:
                    return 0.0
                return ((w / 2 if kind else w) + 65) / 0.96 + 60

            def emit_square(i, st, cols, kind):
                # dst-based square into the rotating bf16 pool; split cols
                # between ACT [0,a) and DVE [a,cols), 512-aligned
                sq = sqp.tile([128, cols], BF16, tag="sqt", bufs=5, name=f"sq_{i}")
                best, best_a = None, 0
                for a in range(0, cols + 1, W):
                    ea = max(act_clk[0], arr[i]) + a_cost(a) if a else act_clk[0]
                    ed = (max(dve_clk[0], arr[i]) + d_cost(cols - a, kind)
                          if a < cols else dve_clk[0])
                    t = max(ea, ed)
                    if best is None or t < best:
                        best, best_a = t, a
                a = best_a
                if a > 0:
                    nc.scalar.activation(sq[:, 0:a], st[:, 0:a], AF.Square)
                    act_clk[0] = max(act_clk[0], arr[i]) + a_cost(a)
                if a < cols:
                    nc.vector.tensor_mul(sq[:, a:cols], st[:, a:cols], st[:, a:cols])
                    dve_clk[0] = max(dve_clk[0], arr[i]) + d_cost(cols - a, kind)
                return sq

            # start=True must fire once per PAIR per bank
            seen_dot = set()
            seen_nrm = set()

            def dot_mms(i):
                b, c0, c1 = STRIPES[i][:3]
                st = stripes[i]
                j = b // 2
                rows = slice(32 * j, 32 * j + 32)
                mms = []
                for t in range(c0 // W, c1 // W):
                    c = NW * (b % 2) + t
                    tl = t - c0 // W
                    first = j not in seen_dot
                    seen_dot.add(j)
                    mms.append((dotPs, rows, strips[:, b, 31 - c : 63 - c],
                                st[:, tl * W : (tl + 1) * W], j, first))
                return mms

            def nrm_mms(i, sq):
                b, c0, c1 = STRIPES[i][:3]
                j = b // 2
                rows = slice(32 * j, 32 * j + 32)
                mms = []
                for t in range(c0 // W, c1 // W):
                    c = NW * (b % 2) + t
                    tl = t - c0 // W
                    first = j not in seen_nrm
                    seen_nrm.add(j)
                    mms.append((nrmPs, rows, ones_strip[:, 31 - c : 63 - c],
                                sq[:, tl * W : (tl + 1) * W], j, first))
                return mms

            def emit_zip(streams, stop_stream=None):
                for k in range(max(len(s) for s in streams)):
                    for si, s in enumerate(streams):
                        if k < len(s):
                            ps, rows, lhsT, rhs, j, first = s[k]
                            stop = si == stop_stream and k == len(s) - 1
                            nc.tensor.matmul(
                                ps[rows, :], lhsT=lhsT, rhs=rhs,
                                start=first, stop=stop,
                                skip_group_check=True,
                                tile_position=(0, 32 * j),
                            )

            # Super-slots: dots of stripes (i, i+1) zipped with norms of
            # stripes (i-2, i-1).  Squares are dst-based so they are emitted
            # BEFORE the zip (start at DMA arrival, no WAR on the dots).
            pending = []  # (stripe_idx, squared tile)
            n_str = len(STRIPES)
            slots_plan = [[0]] + [
                list(range(i, min(i + 2, n_str))) for i in range(1, n_str, 2)
            ]
            for slots in slots_plan:
                i0 = slots[0]
                sqs = {}
                for x in slots:
                    b, c0, c1, kind = STRIPES[x][:4]
                    sqs[x] = emit_square(x, stripes[x], c1 - c0, kind)
                streams = [dot_mms(x) for x in slots]
                stop_stream = None
                if n_str - 1 in slots:
                    stop_stream = slots.index(n_str - 1)
                while pending and len(streams) < 4 and pending[0][0] <= i0 - 1:
                    streams.append(nrm_mms(*pending.pop(0)))
                emit_zip(streams, stop_stream=stop_stream)
                for x in slots:
                    pending.append((x, sqs[x]))
                if i0 == 1:
                    nc.tensor.matmul(absorb[:, 1:2], lhsT=P_sb[:], rhs=ones_col[:],
                                     start=True, stop=True, skip_group_check=True)

            emit_proj_block()
            while pending:
                last = len(pending) == 1
                emit_zip([nrm_mms(*pending.pop(0))], stop_stream=0 if last else None)

            # ---- light phase (critical tail) ----
            # 1/(kn*mn) = exp(-0.5*ln(kn2*mn2)); sim; E = exp(beta*sim)
            Lv = lp.tile([128, W], F32, name="Lv")
            nc.scalar.activation(Lv[:], nrmPs[:], AF.Ln, scale=F_kn2[:])
            y1 = lp.tile([128, W], F32, name="y1")
            nc.scalar.activation(y1[:], Lv[:], AF.Exp, scale=-0.5)
            sim = lp.tile([128, W], F32, name="sim")
            nc.vector.tensor_mul(sim[:], dotPs[:], y1[:])

            E = lp.tile([128, W], F16, name="E")
            rs1 = lp.tile([128, 1], F32, name="rs1")
            nc.scalar.activation(E[:], sim[:], AF.Exp, scale=F_beta, accum_out=rs1[:])
            FS_ps = psB.tile([128, 1], F32, tag="tiny")
            nc.tensor.matmul(FS_ps[:], lhsT=P_sb[:], rhs=rs1[:], start=True, stop=True)

            # CV = g*conv(E) in fp16 (DVE 2x on all-2B operands): starts
            # right after E, does NOT wait on the softmax-sum (FS);
            # SH = CV/S + PWc folds the 1/S at the end.
            EL = lp.tile([128, 1], F16, name="EL")
            ER = lp.tile([128, 1], F16, name="ER")
            nc.vector.stream_shuffle(EL[:], E[:, 0:1], MASK_L)
            nc.vector.stream_shuffle(ER[:], E[:, W - 1 : W], MASK_R)
            CV = lp.tile([128, W], F16, name="CV")
            nc.vector.tensor_scalar_mul(CV[:], E[:], F_gs1[:])
            nc.vector.scalar_tensor_tensor(
                CV[:, 0 : W - 1], E[:, 1:W], F_gs0[:], CV[:, 0 : W - 1],
                op0=OP.mult, op1=OP.add)
            nc.vector.scalar_tensor_tensor(
                CV[:, 1:W], E[:, 0 : W - 1], F_gs2[:], CV[:, 1:W],
                op0=OP.mult, op1=OP.add)
            nc.vector.scalar_tensor_tensor(
                CV[:, W - 1 : W], EL[:], F_gs0[:], CV[:, W - 1 : W],
                op0=OP.mult, op1=OP.add)
            nc.vector.scalar_tensor_tensor(
                CV[:, 0:1], ER[:], F_gs2[:], CV[:, 0:1], op0=OP.mult, op1=OP.add)
            F_invS = lp.tile([128, 1], F32, name="F_invS")
            nc.vector.reciprocal(F_invS[:], FS_ps[:])
            SH = lp.tile([128, W], F16, name="SH")
            nc.vector.scalar_tensor_tensor(
                SH[:], CV[:], F_invS[:], PWc[:], op0=OP.mult, op1=OP.add)

            Lg = lp.tile([128, W], F32, name="Lg")
            nc.scalar.activation(Lg[:], SH[:], AF.Ln, bias=eps_col[:])
            P2 = lp.tile([128, W], F32, name="P2")
            rs2 = lp.tile([128, 1], F32, name="rs2")
            nc.scalar.activation(P2[:], Lg[:], AF.Exp, scale=F_gamma, accum_out=rs2[:])
            S2_ps = psB.tile([128, 1], F32, tag="tiny")
            nc.tensor.matmul(S2_ps[:], lhsT=P_sb[:], rhs=rs2[:], start=True, stop=True)
            S2e = lp.tile([128, 1], F32, name="S2e")
            nc.vector.tensor_scalar_add(S2e[:], S2_ps[:], EPS)
            F_r2 = lp.tile([128, 1], F32, name="F_r2")
            nc.vector.reciprocal(F_r2[:], S2e[:])
            # final normalize on DVE (idle right after the reciprocal --
            # same-engine, no sem hop) and a single out-DMA
            outsb = lp.tile([128, W], F32, name="outsb")
            nc.vector.tensor_scalar_mul(outsb[:], P2[:], F_r2[:])
            nc.sync.dma_start(out_d[:], outsb[:])
    nc.compile()
    return nc


def _get_nc():
    global _NC
    if _NC is None:
        _NC = build_nc()
    return _NC


def _enable_profiling():
    import types

    import concourse.bass_utils as bu

    bu.upload_artifacts = lambda tmpdir: tmpdir
    try:
        from antenv.axon_hooks import get_axon_ntff_profile_hook  # noqa: F401

        return
    except ImportError:
        pass
    import antenv

    mod = types.ModuleType("antenv.axon_hooks")
    _holder = {}
    mod.set_axon_ntff_profile_hook = lambda h: _holder.__setitem__("h", h)
    mod.get_axon_ntff_profile_hook = lambda: _holder.get("h")
    sys.modules["antenv.axon_hooks"] = mod
    antenv.axon_hooks = mod
    from trn_agent_boot.trn_boot import _ntff_profile_via_ctypes

    mod.set_axon_ntff_profile_hook(
        _ntff_profile_via_ctypes("/opt/axon/libaxon_pjrt.so")
    )


def _host_pack(inputs, core):
    cs = np.ascontiguousarray(np.asarray(inputs["controller_state"], dtype=np.float32))
    Wk = np.asarray(inputs["Wk"], np.float32)
    Wcat = np.concatenate(
        [
            np.asarray(inputs["Wb"], np.float32),
            np.asarray(inputs["Wg"], np.float32),
            np.asarray(inputs["Ws"], np.float32),
            np.asarray(inputs["Wgam"], np.float32),
        ],
        axis=1,
    )
    brow = np.concatenate(
        [
            np.asarray(inputs["bb"], np.float32),
            np.asarray(inputs["bg"], np.float32),
            np.asarray(inputs["bs"], np.float32),
            np.asarray(inputs["bgam"], np.float32),
        ]
    )
    csT = cs.reshape(NCORES, BL, C).transpose(0, 2, 1)  # (cores, C, BL)

    pack = np.zeros((128, PACK_W), np.float32)

    def put(name, val, rows=128):
        a, bb_ = PACK_COLS[name]
        pack[0:rows, a:bb_] = val

    put("csT0", csT[core][0:128, :])
    put("csT1", csT[core][128:256, :])
    put("Wk0", Wk[0:128, :])
    put("Wk1", Wk[128:256, :])
    put("Wc0", Wcat[0:128, :])
    put("Wc1", Wcat[128:256, :])
    ind = np.zeros((128, BL), np.float32)
    for p in range(128):
        ind[p, p // NW] = 1.0
    put("indT", np.zeros((BL, 128), np.float32) + ind.T, rows=BL)
    put("ones_col", np.ones((128, 1), np.float32))
    ones_strip = np.zeros((128, 63), np.float32)
    ones_strip[:, 31] = 1.0
    put("ones_strip", ones_strip)
    put("P", np.kron(np.eye(BL, dtype=np.float32), np.ones((NW, NW), np.float32)))
    put("bias6", np.broadcast_to(brow[None, :], (BL, 6)), rows=BL)
    put("eps_col", np.full((128, 1), EPS, np.float32))
    return (np.ascontiguousarray(pack[:, :PACKA_W]),
            np.ascontiguousarray(pack[:, PACKA_W:]))


def kernel(**inputs):
    global LAST_RESULTS
    import ml_dtypes

    mem = np.ascontiguousarray(np.asarray(inputs["memory"], dtype=np.float32))
    pw = np.ascontiguousarray(np.asarray(inputs["previous_weights"], dtype=np.float32))

    # shard: core c gets batches [c*BL, (c+1)*BL); memory pre-transposed
    memT = mem.reshape(NCORES, BL, N, D).transpose(0, 1, 3, 2)  # [core,b,D,N]
    pw_sh = pw.reshape(NCORES, BL, N)

    packs = [_host_pack(inputs, c) for c in range(NCORES)]
    # (batch, half) per m8/mb slot, in STRIPES src_idx order
    f8_order = [(0, 0), (2, 0), (4, 0), (1, 0), (3, 0),
                (0, 1), (2, 1), (4, 1), (1, 1), (3, 1), (7, 1)]
    bf_order = [(6, 0), (5, 0), (7, 0), (6, 1), (5, 1)]
    in_maps = []
    for c in range(NCORES):
        m8 = np.stack(
            [memT[c, b, :, h * H:(h + 1) * H] for b, h in f8_order]
        ).astype(ml_dtypes.float8_e3m4)
        mb = np.stack(
            [memT[c, b, :, h * H:(h + 1) * H] for b, h in bf_order]
        ).astype(ml_dtypes.bfloat16)
        in_maps.append({
            "m8": np.ascontiguousarray(m8),
            "mb": np.ascontiguousarray(mb),
            "packA": packs[c][0],
            "packB": packs[c][1],
            "pw": np.ascontiguousarray(pw_sh[c].reshape(128, W).astype(np.float16)),
        })
    nc = _get_nc()
    if PROFILE:
        _enable_profiling()
    res = run_bass_kernel_spmd(nc, in_maps, list(range(NCORES)), trace=PROFILE)
    LAST_RESULTS = res
    out = np.concatenate(
        [r["out"].reshape(BL, N) for r in res.results], axis=0
    )
    return out.astype(np.float32)
